# revision 5
# baseline (speedup 1.0000x reference)
# Trainium2 Bass kernel for DenseBipartiteGAT (B=8, N=1024, C=256, H=4, D=64).
#
# Math: scores[t,s,h] = lrelu(a_tgt[t,h] + a_src[s,h], 0.2), masked softmax over s,
#       out[t] = sum_s attn * h_src.
# Factorization: exp(lrelu(u+v)) = e^u e^v if u+v>=0 else e^.2u e^.2v. Dividing
# num/den cancels e^{.2u}, leaving r=e^{.8u}:
#   out = (r*A1 + (G - A2)) / (r*A1d + (G - A2)d + eps)
# with A1 = sum_s m1*F1*hsrc, A2 = sum_s m1*F2*hsrc, G = sum_s em*F2*hsrc,
# m1 = em*[u+v>=0], F1 = e^v*mask_s, F2 = e^{.2v}*mask_s.
#
# Rewrite with the complement mask mx = em*[u+v<0] (m1 = em - mx):
#   A1     = em@R1 - mx@R1
#   G - A2 = mx@R2
# Using signed encodings ptxn = -[u<-a], mxn = ptxn*em (0/-1), em (0/1) and
# signed R-regions R1p = F1*hsrc, R2n = -F2*hsrc (den cols F1 / -F2), a single
# PSUM tile [j,h,65] accumulates  j0: em@R1p + mxn@R1p = A1(+den1),
#                                 j1: mxn@R2n = (G-A2)(+den).
# Combine: W = r*psm[j0] + psm[j1]; out = W[:, :64] * mask/(W[:,64]+eps) + bias.
#
# Per-core O(N^2) vector work: em (1 op/st), ptxn (1 op/(st,h)), mxn (1 op/(st,h)).
# Everything else is PE matmuls (f32r for the fp32 input projections).
#
# Sharding: data-parallel over batch B across the 8 cores (1 batch element each).

import hashlib
import os
import shutil

import numpy as np

B, N, C, H, D = 8, 1024, 256, 4, 64
NT = N // 128  # 8 tiles of 128 along s or t
EPS = 1e-12

_CACHED = {}


def _install_neff_cache():
    """Content-addressed NEFF cache: walrus compile is ~8min, cache by BIR hash."""
    import concourse.bass2jax as b2j
    import concourse.bass_utils as bu

    if getattr(b2j, "_neff_cache_installed", False):
        return
    cache_dir = os.environ.get("NEFF_CACHE_DIR", "/tmp/neff_cache")
    os.makedirs(cache_dir, exist_ok=True)
    orig = bu.compile_bir_kernel

    def cached_compile(bir_json: bytes, tmpdir: str, neff_name="file.neff") -> str:
        key = hashlib.sha256(bir_json).hexdigest()
        cpath = os.path.join(cache_dir, f"{key}.neff")
        opath = os.path.join(tmpdir, neff_name)
        if os.path.exists(cpath):
            shutil.copy(cpath, opath)
            return opath
        neff = orig(bir_json, tmpdir, neff_name)
        try:
            shutil.copy(neff, cpath)
        except OSError:
            pass
        return neff

    bu.compile_bir_kernel = cached_compile
    b2j.compile_bir_kernel = cached_compile
    b2j._neff_cache_installed = True


def build_nc(reps=1):
    """Build the Bass program (one core's work; SPMD across 8 cores).

    reps>1 emits the body multiple times (timing: marginal-cost slope)."""
    import concourse.tile as tile
    import concourse.mybir as mybir
    from concourse import bacc
    from concourse.bass import ts, ds

    f32 = mybir.dt.float32
    f32r = mybir.dt.float32r
    f16 = mybir.dt.float16
    Alu = mybir.AluOpType
    Act = mybir.ActivationFunctionType

    nc = bacc.Bacc("TRN2", target_bir_lowering=False, debug=False, num_devices=B)

    xsT = nc.dram_tensor("xsT", (C, N), f16, kind="ExternalInput").ap()
    xtT = nc.dram_tensor("xtT", (C, N), f16, kind="ExternalInput").ap()
    adjTh = nc.dram_tensor("adjTh", (N, N), f16, kind="ExternalInput").ap()
    maskp = nc.dram_tensor("maskp", (128, NT), f32, kind="ExternalInput").ap()
    maskln = nc.dram_tensor("maskln", (128, NT), f32, kind="ExternalInput").ap()
    wes = nc.dram_tensor("wes", (C, 260), f16, kind="ExternalInput").ap()
    wbt = nc.dram_tensor("wbt", (C, 4), f16, kind="ExternalInput").ap()
    biasrow = nc.dram_tensor("biasrow", (1, 256), f32, kind="ExternalInput").ap()
    out = nc.dram_tensor("out", (N, 256), f32, kind="ExternalOutput").ap()

    with tile.TileContext(nc) as tc:
        with (
            tc.tile_pool(name="singles", bufs=1) as singles,
            tc.tile_pool(name="pch", bufs=8, space="PSUM") as pch,
            tc.tile_pool(name="adjs", bufs=3) as adj_pool,
            tc.tile_pool(name="emp", bufs=NT) as em_pool,
            tc.tile_pool(name="rp", bufs=NT) as r_pool,
            tc.tile_pool(name="fx", bufs=2) as f_pool,
            tc.tile_pool(name="ab", bufs=NT) as a_pool,
            tc.tile_pool(name="mx", bufs=4 * NT) as mx_pool,
            tc.tile_pool(name="ptp", bufs=3) as pt_pool,
            tc.tile_pool(name="wt", bufs=4) as w_pool,
            tc.tile_pool(name="outs", bufs=2) as out_pool,
            tc.tile_pool(name="dram", bufs=1, space="DRAM") as dram_pool,
        ):
          for _rep in range(reps):
            # ---- loads. Act queue: xtT only (u critical path); SP: the rest+adjT;
            # gpsimd: u roundtrip + bias.
            xtT_sb = singles.tile([128, 2, N], f16)
            nc.scalar.dma_start(xtT_sb, xtT.rearrange("(ko p) n -> p ko n", p=128))
            xsT_sb = singles.tile([128, 2, N], f16)
            nc.sync.dma_start(xsT_sb, xsT.rearrange("(ko p) n -> p ko n", p=128))
            wes_sb = singles.tile([128, 2, 260], f16)
            nc.sync.dma_start(wes_sb, wes.rearrange("(ko p) n -> p ko n", p=128))
            wbt_sb = singles.tile([128, 2, 4], f16)
            nc.scalar.dma_start(wbt_sb, wbt.rearrange("(ko p) n -> p ko n", p=128))
            maskp_sb = singles.tile([128, NT], f32)
            nc.sync.dma_start(maskp_sb, maskp)
            maskln_sb = singles.tile([128, NT], f32)
            nc.sync.dma_start(maskln_sb, maskln)
            bias_bc = singles.tile([128, 256], f32)
            nc.gpsimd.dma_start(bias_bc, biasrow.broadcast_to([128, 256]))

            # ---- u^T = wbt^T @ xtT in 256-col chunks -> DRAM -> broadcast ----
            u_sb = singles.tile([4, N], f16)
            for q in range(4):
                psu = pch.tile([128, 6, 65], f32, tag="px", bufs=8, name="psu")
                psuf = psu.rearrange("p a b -> p (a b)")
                for ko in range(2):
                    nc.tensor.matmul(
                        psuf[0:4, 0:256],
                        lhsT=wbt_sb[:, ko, :],
                        rhs=xtT_sb[:, ko, ds(q * 256, 256)],
                        start=(ko == 0),
                        stop=(ko == 1),
                    )
                nc.scalar.activation(
                    u_sb[:, q * 256 : (q + 1) * 256], psuf[0:4, 0:256], Act.Identity
                )
            u_dram = dram_pool.tile([4, N], f16)
            nc.gpsimd.dma_start(u_dram, u_sb)
            u_bc = singles.tile([128, 4, N], f16)
            ones_sb = singles.tile([1, 128], f16)
            nc.vector.memset(ones_sb, 1.0)
            for q in range(4):
                psb = pch.tile([128, 6, 65], f32, tag="px", bufs=8, name="psb")
                psbf = psb.rearrange("p a b -> p (a b)")
                nc.tensor.matmul(
                    psbf[:, 0:256],
                    lhsT=ones_sb,
                    rhs=u_sb[0:1, ds(q * 256, 256)],
                    start=True,
                    stop=True,
                )
                nc.scalar.activation(
                    u_bc[:, 0, q * 256 : (q + 1) * 256], psbf[:, 0:256], Act.Identity
                )

            # ---- phase A per s-tile: hsrc matmul -> F exps -> R build -> a_neg ----
            R_tiles = []
            an_tiles = []
            for st in range(NT):
                psx = pch.tile([128, 6, 65], f32, tag="px", bufs=8, name="psx")
                ps = psx.rearrange("p a b -> p (a b)")[:, 0:260]
                for ko in range(2):
                    nc.tensor.matmul(
                        ps,
                        lhsT=xsT_sb[:, ko, ts(st, 128)],
                        rhs=wes_sb[:, ko, :],
                        start=(ko == 0),
                        stop=(ko == 1),
                    )
                lnm = maskln_sb[:, st : st + 1]
                # a_neg = -a_src (scalar operand of the ptxn compare)
                an = a_pool.tile([128, 4], f32, name=f"an{st}", tag="an")
                nc.scalar.activation(an, ps[:, 256:260], Act.Identity, scale=-1.0)
                an_tiles.append(an)
                Fx = f_pool.tile([128, 2, 4], f32, tag="fx")
                nc.scalar.activation(Fx[:, 0, :], ps[:, 256:260], Act.Exp, bias=lnm)
                nc.scalar.activation(
                    Fx[:, 1, :], ps[:, 256:260], Act.Exp, bias=lnm, scale=0.2
                )
                # negate F2 in place (R2 region is stored negated)
                nc.vector.tensor_scalar(Fx[:, 1, :], Fx[:, 1, :], -1.0, None, Alu.mult)
                R = r_pool.tile([128, 2, 4, 65], f16, name=f"R{st}", tag="R")
                ps4 = ps[:, 0:256].rearrange("p (h d) -> p h d", h=4)
                if st % 2 == 0:
                    for j in range(2):
                        nc.vector.tensor_tensor(
                            R[:, j, :, 0:64],
                            ps4,
                            Fx[:, j, :].unsqueeze(2).broadcast_to([128, 4, 64]),
                            Alu.mult,
                        )
                else:
                    for j in range(2):
                        for h4 in range(4):
                            nc.scalar.activation(
                                R[:, j, h4, 0:64], ps4[:, h4, :], Act.Identity,
                                scale=Fx[:, j, h4 : h4 + 1],
                            )
                nc.vector.tensor_copy(out=R[:, :, :, 64], in_=Fx)
                R_tiles.append(R)

            # ---- r_sb[t_part, h] = exp(0.8 * a_tgt) per t-tile ----
            r_sb_tiles = []
            for tt in range(NT):
                psr = pch.tile([128, 6, 65], f32, tag="px", bufs=8, name="psr")
                psrf = psr.rearrange("p a b -> p (a b)")
                for ko in range(2):
                    nc.tensor.matmul(
                        psrf[:, 0:4],
                        lhsT=xtT_sb[:, ko, ts(tt, 128)],
                        rhs=wbt_sb[:, ko, :],
                        start=(ko == 0),
                        stop=(ko == 1),
                    )
                r_sb = a_pool.tile([128, 4], f32, name=f"rsb{tt}", tag="rsb")
                nc.scalar.activation(r_sb, psrf[:, 0:4], Act.Exp, scale=0.8)
                r_sb_tiles.append(r_sb)

            # ---- adjT loads (SP queue; u_bc h1-3 DMA slotted after #2) ----
            adjT_tiles = []
            for st in range(NT):
                adjT = adj_pool.tile([128, N], f16, tag="adjT", bufs=NT)
                nc.sync.dma_start(adjT, adjTh[ts(st, 128), :])
                adjT_tiles.append(adjT)
                if st == 2:
                    # heads 1-3 of u broadcast in one DMA (h0 via PE above)
                    nc.sync.dma_start(
                        u_bc[:, 1:4, :],
                        u_dram[1:4, :].unsqueeze(0).broadcast_to([128, 3, N]),
                    )

            # ---- em (0/1) + masks mxn = -[u < -a] * em  (0/-1) ----
            # DVE order: em 0-2, ptxn h0, em 3-7, then per-h mask stream.
            em_tiles = [None] * NT

            def emit_em(st):
                # emN in {0,-1}: -1 on edges
                em = em_pool.tile([128, N], f16, name=f"em{st}", tag="em")
                nc.vector.tensor_scalar(
                    em, adjT_tiles[st], 0.0, -1.0, Alu.not_equal, Alu.mult
                )
                em_tiles[st] = em

            mx_tiles = [[None] * 4 for _ in range(NT)]

            def on_pool(st, h):
                return (st * 4 + h) % 2 == 0

            def emit_ptx(st, h):
                ptx = pt_pool.tile([128, N], f16, tag="ptx", bufs=12)
                nc.vector.tensor_scalar(
                    ptx, u_bc[:, h, :], an_tiles[st][:, h : h + 1], None, Alu.is_lt
                )
                return ptx

            def emit_mx(st, h, ptx):
                mx = mx_pool.tile([128, N], f16, name=f"mx{st}_{h}", tag="mx")
                eng = nc.gpsimd if on_pool(st, h) else nc.vector
                eng.tensor_tensor(mx, ptx, em_tiles[st], Alu.mult)
                mx_tiles[st][h] = mx

            for st in range(3):
                emit_em(st)
            ptx_h0 = [emit_ptx(st, 0) for st in range(NT)]
            for st in range(3, NT):
                emit_em(st)
            for st in range(NT):
                emit_mx(st, 0, ptx_h0[st])

            # ---- chains: head-pair two-phase over all 8 t-tiles ----
            # px tile per (t, head-pair): [ha:j0,j1 | hb:j0,j1 | emE ha,hb]
            # j0 -= mx@R1p, j1 = mx@R2, emE = em@R1p
            pX_tiles = [None] * NT
            W_all = singles.tile([128, NT, 4, 65], f32)
            bmask_all = singles.tile([128, NT, 256], f32)
            nc.gpsimd.tensor_tensor(
                bmask_all,
                bias_bc.unsqueeze(1).broadcast_to([128, NT, 256]),
                maskp_sb.unsqueeze(2).broadcast_to([128, NT, 256]),
                Alu.mult,
            )

            def emit_em_chain(t, hp):
                pX = pch.tile([128, 6, 65], f32, name=f"psm{hp}_{t}", tag="px", bufs=8)
                pX_tiles[t] = pX
                for st in range(NT):
                    nc.tensor.matmul(
                        pX[:, 4:6, :].rearrange("p a b -> p (a b)"),
                        lhsT=em_tiles[st][:, ts(t, 128)],
                        rhs=R_tiles[st][:, 0, 2 * hp : 2 * hp + 2, :],
                        start=(st == 0),
                        stop=False,
                    )

            def emit_mx_chain(t, h):
                pX = pX_tiles[t]
                for st in range(NT):
                    nc.tensor.matmul(
                        pX[:, 2 * (h % 2) : 2 * (h % 2) + 2, :].rearrange(
                            "p a b -> p (a b)"
                        ),
                        lhsT=mx_tiles[st][h][:, ts(t, 128)],
                        rhs=R_tiles[st][:, :, h, :],
                        start=False,
                        stop=(st == NT - 1 and h % 2 == 1),
                        skip_group_check=True,
                    )

            def emit_W(t, hp):
                # Act evacuates the px tile to SBUF (GPSIMD can't touch PSUM),
                # then W[ha,hb] = r*(j0 - emE) + mx_j1 on Pool
                W = W_all[:, t]
                pX = pX_tiles[t]
                pxe = w_pool.tile([128, 6, 65], f32, tag="pxe", bufs=4)
                nc.scalar.activation(
                    pxe.rearrange("p a b -> p (a b)"),
                    pX.rearrange("p a b -> p (a b)"),
                    Act.Identity,
                )
                for i in range(2):
                    h = 2 * hp + i
                    nc.gpsimd.tensor_tensor(
                        W[:, h, :], pxe[:, 2 * i, :], pxe[:, 4 + i, :], Alu.subtract
                    )
                    nc.gpsimd.tensor_tensor(
                        W[:, h, :], W[:, h, :],
                        r_sb_tiles[t][:, h : h + 1].broadcast_to([128, 65]),
                        Alu.mult,
                    )
                    nc.gpsimd.tensor_tensor(
                        W[:, h, :], W[:, h, :], pxe[:, 2 * i + 1, :], Alu.add
                    )

            def emit_finals():
                dent = w_pool.tile([128, NT, 4], f32, tag="dent")
                nc.vector.tensor_scalar(dent, W_all[:, :, :, 64], EPS, None, Alu.add)
                nc.vector.reciprocal(dent, dent)
                nc.vector.tensor_tensor(
                    dent, dent,
                    maskp_sb.unsqueeze(2).broadcast_to([128, NT, 4]), Alu.mult,
                )
                ot = out_pool.tile([128, NT, 256], f32, tag="ot", bufs=1)
                ov = ot.rearrange("p t (h d) -> p t h d", h=4)
                nc.vector.tensor_tensor(
                    ov[:, 0 : NT // 2], W_all[:, 0 : NT // 2, :, 0:64],
                    dent[:, 0 : NT // 2].unsqueeze(3).broadcast_to(
                        [128, NT // 2, 4, 64]
                    ),
                    Alu.mult,
                )
                nc.gpsimd.tensor_tensor(
                    ov[:, NT // 2 :], W_all[:, NT // 2 :, :, 0:64],
                    dent[:, NT // 2 :].unsqueeze(3).broadcast_to(
                        [128, NT // 2, 4, 64]
                    ),
                    Alu.mult,
                )
                nc.gpsimd.tensor_tensor(ot, ot, bmask_all, Alu.add)
                nc.sync.dma_start(
                    out.rearrange("(t p) c -> p t c", p=128), ot
                )

            for t in range(NT):
                emit_em_chain(t, 0)
            for st in range(NT):
                emit_mx(st, 1, emit_ptx(st, 1))
            for t in range(NT):
                emit_mx_chain(t, 0)
            for t in range(NT):
                emit_mx_chain(t, 1)
            for t in range(NT):
                emit_W(t, 0)
            for t in range(NT):
                emit_em_chain(t, 1)
            for st in range(NT):
                emit_mx(st, 2, emit_ptx(st, 2))
            for t in range(NT):
                emit_mx_chain(t, 2)
            for st in range(NT):
                emit_mx(st, 3, emit_ptx(st, 3))
            for t in range(NT):
                emit_mx_chain(t, 3)
            for t in range(NT):
                emit_W(t, 1)
            emit_finals()

    nc.compile()
    return nc


def host_prep(x_source, x_target, adj, mask, W_src, W_tgt, att_src, att_tgt, bias):
    """Per-core input maps (layout prep only: transposes / views / weight folding)."""
    x_source = np.asarray(x_source, dtype=np.float32)
    x_target = np.asarray(x_target, dtype=np.float32)
    adj = np.ascontiguousarray(np.asarray(adj, dtype=np.float32))
    mask = np.asarray(mask)
    W_src = np.asarray(W_src, dtype=np.float32)
    W_tgt = np.asarray(W_tgt, dtype=np.float32)
    att_src = np.asarray(att_src, dtype=np.float32)
    att_tgt = np.asarray(att_tgt, dtype=np.float32)
    bias = np.asarray(bias, dtype=np.float32)

    w_a = np.einsum(
        "hdc,hd->ch", W_src.astype(np.float64).reshape(H, D, C), att_src.astype(np.float64)
    ).astype(np.float32)
    w_b = np.einsum(
        "hdc,hd->ch", W_tgt.astype(np.float64).reshape(H, D, C), att_tgt.astype(np.float64)
    ).astype(np.float32)
    wes = np.ascontiguousarray(np.concatenate([W_src.T, w_a], axis=1).astype(np.float16))  # (256, 260)
    wbt = np.ascontiguousarray(w_b.astype(np.float16))  # (256, 4)
    biasrow = np.ascontiguousarray(bias.reshape(1, 256))

    in_maps = []
    for b in range(B):
        mb = mask[b].astype(np.float32)
        maskp = mb.reshape(NT, 128).T.copy()  # (128, NT), p-inner
        maskln = np.where(mb > 0, 0.0, -60.0).astype(np.float32).reshape(NT, 128).T.copy()
        # hi 2 bytes of each f32: zero iff (virtually certainly) adj == 0.
        adjTh = np.ascontiguousarray(adj[b].view(np.float16)[:, 1::2].T)  # (Ns, Nt)
        in_maps.append(
            {
                "xsT": np.ascontiguousarray(x_source[b].T.astype(np.float16)),
                "xtT": np.ascontiguousarray(x_target[b].T.astype(np.float16)),
                "adjTh": adjTh,
                "maskp": maskp,
                "maskln": maskln,
                "wes": wes,
                "wbt": wbt,
                "biasrow": biasrow,
            }
        )
    return in_maps


def get_nc():
    if "nc" not in _CACHED:
        _install_neff_cache()
        _CACHED["nc"] = build_nc()
    return _CACHED["nc"]


def kernel(**inputs) -> np.ndarray:
    from concourse.bass_utils import run_bass_kernel_spmd

    nc = get_nc()
    in_maps = host_prep(**inputs)
    res = run_bass_kernel_spmd(nc, in_maps, core_ids=list(range(B)))
    return np.stack([r["out"] for r in res.results]).astype(np.float32)
